# revision 1
# baseline (speedup 1.0000x reference)
"""Trainium2 Bass kernel for nn_PointTransformerLayer (N=1024, dim=64, 8 cores).

Sharding: query rows i are split across 8 cores (128 rows each, flash-attention
style); k/v/pos and all weights are replicated to every core host-side.

Math (per core, rows i in its slice, all j):
  a_i  = p_i @ W_pos1 + b_pos1            (per-i, precomputed)
  bn_j = -(p_j @ W_pos1)                  (per-j, precomputed)
  R    = relu(a_i + bn_j)                 -> bf16                    [pos MLP l1]
  U    = R @ W_pos2 - k_j + q_i + b_pos2  (k folded into the matmul via -I rows,
                                           q_i + b_pos2 added as evac bias)
  H    = relu(U @ W_attn1 + b_attn1)      -> bf16
  sim  = H @ W_attn2                      (b_attn2 dropped: softmax-invariant)
  E    = exp(sim)  (no max-sub: |sim| < ~1)
  agg  = (sum_j E*U + sum_j E*(v_j+k_j)) / sum_j E - q_i
       == softmax(sim) . (v_j + rpe)  since v_e = U + (v+k) - q

Layouts are feature-major: [features on partitions, points on free dim].
Two query rows are processed per iteration by packing their 64-wide feature
tensors into the 128 partitions (group A = rows 0..63, group B = rows 64..127
of the core's slice).
"""

import sys

sys.path.insert(0, "/opt/trn_rl_repo")

import numpy as np
import ml_dtypes

import concourse.bass as bass
import concourse.bacc as bacc
import concourse.mybir as mybir
import concourse.tile as tile
from concourse.bass_utils import run_bass_kernel_spmd
from concourse.tile_rust import add_dep_helper

F32 = mybir.dt.float32
BF16 = mybir.dt.bfloat16
AF = mybir.ActivationFunctionType
OP = mybir.AluOpType

N = 1024
DIM = 64
HID = 256  # DIM * ATTN_MULT
NCORES = 8
ROWS = N // NCORES  # 128 query rows per core
NPAIR = ROWS // 2  # 64 iterations, 2 rows (A/B groups) each

_CACHE = {}
_CONFIG = {}


def _setup_phase(nc, tc, t):
    """DMA loads + small precompute matmuls. Fills the persistent tiles in t."""
    with tc.tile_pool(name="setup_ps", bufs=2, space="PSUM") as spool:
        # k^T, v^T over all j  (f32 matmuls)
        for jc in range(2):
            ps = spool.tile([DIM, 512], F32, tag="kv_ps", name="ps")
            nc.tensor.matmul(ps[:], t.Wk[:], t.xT[:, jc * 512:(jc + 1) * 512])
            nc.scalar.copy(t.tmp_kT[:, jc * 512:(jc + 1) * 512], ps[:])
            ps2 = spool.tile([DIM, 512], F32, tag="kv_ps", name="ps2")
            nc.tensor.matmul(ps2[:], t.Wv[:], t.xT[:, jc * 512:(jc + 1) * 512])
            nc.scalar.copy(t.tmp_vT[:, jc * 512:(jc + 1) * 512], ps2[:])
        # vk = v + k (bf16, both partition halves)
        nc.vector.tensor_tensor(
            out=t.vk2[0:DIM, :], in0=t.tmp_kT[:], in1=t.tmp_vT[:], op=OP.add
        )
        nc.vector.tensor_copy(t.vk2[DIM:128, :], t.vk2[0:DIM, :])
        # static k^T (bf16) into partitions 64:128 of all four R buffers
        for RA, RB in t.Rbufs:
            nc.vector.tensor_copy(RA[DIM:128, :], t.tmp_kT[:])
            nc.vector.tensor_copy(RB[DIM:128, :], t.tmp_kT[:])
        # bn = -(p@Wpos1) over all j (bf16 both halves)
        for jc in range(2):
            ps = spool.tile([DIM, 512], F32, tag="bt_ps", name="ps")
            nc.tensor.matmul(ps[:], t.Wpos1[:], t.pT[:, jc * 512:(jc + 1) * 512])
            nc.scalar.activation(
                t.bn2[0:DIM, jc * 512:(jc + 1) * 512], ps[:], AF.Identity,
                bias=0.0, scale=-1.0,
            )
        nc.vector.tensor_copy(t.bn2[DIM:128, :], t.bn2[0:DIM, :])
        # a-cols for this core's rows: a = p_i@Wpos1 + b_pos1
        aps = spool.tile([DIM, ROWS], F32, tag="a_ps", name="aps")
        nc.tensor.matmul(aps[:], t.Wpos1[:], t.pTs[:])
        nc.scalar.activation(t.aA[:], aps[:, 0:NPAIR], AF.Identity, bias=t.bpos1[:])
        nc.scalar.activation(t.aB[:], aps[:, NPAIR:ROWS], AF.Identity, bias=t.bpos1[:])
        # q-cols for this core's rows
        qps = spool.tile([DIM, ROWS], F32, tag="q_ps", name="qps")
        nc.tensor.matmul(qps[:], t.Wq[:], t.xTs[:])
        nc.scalar.copy(t.qT2[0:DIM, :], qps[:, 0:NPAIR])
        nc.scalar.copy(t.qT2[DIM:128, :], qps[:, NPAIR:ROWS])
        nc.scalar.activation(t.qb2[0:DIM, :], qps[:, 0:NPAIR], AF.Identity, bias=t.bpos2[:])
        nc.scalar.activation(t.qb2[DIM:128, :], qps[:, NPAIR:ROWS], AF.Identity, bias=t.bpos2[:])


def _pair_iteration(nc, t, pools, m):
    """One iteration: two query rows (groups A/B) against all 1024 j."""
    wpool, hpool, upool, hpspool, simpool = pools
    RA, RB = t.Rbufs[m % len(t.Rbufs)]
    # R = relu(a_i + bn_j) -> bf16 (R can run ahead of the pipeline, so a
    # slower engine off the critical path is fine here)
    r_eng = nc.gpsimd if _CONFIG.get("r_pool", False) else nc.vector
    r_eng.tensor_scalar(
        out=RA[0:DIM, :], in0=t.bn2[0:DIM, :],
        scalar1=t.aA[:, m:m + 1], scalar2=0.0, op0=OP.add, op1=OP.max,
    )
    r_eng.tensor_scalar(
        out=RB[0:DIM, :], in0=t.bn2[0:DIM, :],
        scalar1=t.aB[:, m:m + 1], scalar2=0.0, op0=OP.add, op1=OP.max,
    )
    # U = [Wpos2; -I]^T @ [R; kT]  (single-mm groups)
    U_sb = wpool.tile([128, N], BF16, tag="U_sb", name="U_sb")
    if _CONFIG.get("split_u", True):
        # per-jc 1-bank tiles, bufs=2: m+1 can start before m's second evac
        for jc in range(2):
            U_ps = upool.tile([128, 512], F32, tag="U_ps", name="U_ps")
            for g, Rt in ((0, RA), (1, RB)):
                nc.tensor.matmul(
                    U_ps[g * DIM:(g + 1) * DIM, :],
                    t.Wp2I[:],
                    Rt[:, jc * 512:(jc + 1) * 512],
                    tile_position=(0, g * DIM),
                )
            nc.scalar.activation(
                U_sb[:, jc * 512:(jc + 1) * 512], U_ps[:], AF.Identity,
                bias=t.qb2[:, m:m + 1],
            )
    else:
        U_ps = upool.tile([128, N], F32, tag="U_ps", name="U_ps", bufs=1)
        for jc in range(2):
            for g, Rt in ((0, RA), (1, RB)):
                nc.tensor.matmul(
                    U_ps[g * DIM:(g + 1) * DIM, jc * 512:(jc + 1) * 512],
                    t.Wp2I[:],
                    Rt[:, jc * 512:(jc + 1) * 512],
                    tile_position=(0, g * DIM),
                )
        nc.scalar.activation(
            U_sb[:], U_ps[:], AF.Identity, bias=t.qb2[:, m:m + 1]
        )
    # H = relu(U @ W_attn1 + b_attn1) -> bf16
    # one [128,1024] PSUM tile per (hb, jc): groups A/B in free-dim halves,
    # so the whole tile shares one per-partition bias (b_attn1[hb]) and the
    # evac is a single FD-1024 op. 2 evacs on ACT, 2 on DVE.
    H_sbs = {}
    for hb in range(2):
        for jc in range(2):
            H_ps = hpspool.tile([128, 2 * 512], F32, tag="H_ps", name="H_ps")
            for g in range(2):
                nc.tensor.matmul(
                    H_ps[:, g * 512:(g + 1) * 512],
                    t.W1[g * DIM:(g + 1) * DIM, hb * 128:(hb + 1) * 128],
                    U_sb[g * DIM:(g + 1) * DIM, jc * 512:(jc + 1) * 512],
                    tile_position=(g * DIM, 0),
                )
            H_sb = hpool.tile([128, 2 * 512], BF16, tag="H_sb", name="H_sb")
            # balance: ACT gets 2 of 4 evacs on even iterations, 3 on odd
            use_act = (jc == 0) or (m % 2 == 1 and hb == 1)
            if use_act:
                nc.scalar.activation(
                    H_sb[:], H_ps[:], AF.Relu, bias=t.b1[:, hb:hb + 1]
                )
            else:
                nc.vector.tensor_scalar(
                    out=H_sb[:], in0=H_ps[:],
                    scalar1=t.b1[:, hb:hb + 1], scalar2=0.0,
                    op0=OP.add, op1=OP.max,
                )
            H_sbs[(hb, jc)] = H_sb
    # sim = H @ W_attn2  (2-mm accumulation chains; keep each PSUM bank's
    # chains strictly sequential: group A completes before group B starts).
    # One 1-bank tile per jc half + per-jc exp lets PE run ahead of ACT.
    EP = wpool.tile([128, 2 * N], BF16, tag="EP", name="EP")
    if _CONFIG.get("split_sim", True):
        sim_tiles = [
            simpool.tile([128, 512], F32, tag="SIM_ps", name="SIM_ps")
            for _ in range(2)
        ]
    else:
        big = simpool.tile([128, N], F32, tag="SIM_ps", name="SIM_ps", bufs=1)
        sim_tiles = [big[:, 0:512], big[:, 512:1024]]
    for jc in range(2):
        SIM_ps = sim_tiles[jc]
        prev_last = None
        for g in range(2):
            insts = []
            for hb in range(2):
                inst = nc.tensor.matmul(
                    SIM_ps[g * DIM:(g + 1) * DIM, :],
                    t.W2[:, hb * DIM:(hb + 1) * DIM],
                    H_sbs[(hb, jc)][:, g * 512:(g + 1) * 512],
                    start=(hb == 0),
                    stop=(hb == 1),
                    tile_position=(0, g * DIM),
                )
                insts.append(inst)
            if prev_last is not None:
                add_dep_helper(
                    insts[0].ins, prev_last.ins, False,
                    "psum zero-region chain order",
                )
            prev_last = insts[1]
        if _CONFIG.get("split_sim", True):
            nc.scalar.activation(
                EP[:, jc * 512:(jc + 1) * 512], SIM_ps[:], AF.Exp
            )
    if not _CONFIG.get("split_sim", True):
        nc.scalar.activation(EP[:, 0:N], big[:], AF.Exp)
    V_sb = wpool.tile([128, N], BF16, tag="V_sb", name="V_sb")
    v_eng = nc.gpsimd if _CONFIG.get("v_pool", False) else nc.vector
    p_eng = nc.gpsimd if _CONFIG.get("p_pool", False) else nc.vector
    v_eng.tensor_tensor(out=V_sb[:], in0=U_sb[:], in1=t.vk2[:], op=OP.add)
    p_eng.tensor_tensor(
        out=EP[:, N:2 * N], in0=EP[:, 0:N], in1=V_sb[:], op=OP.mult
    )
    # two pairwise folds at 2x, then one fused reduce over {E, P} segments:
    # SS[:, 2m] = sum_j E, SS[:, 2m+1] = sum_j P
    ep2 = EP.rearrange("p (k n) -> p k n", k=2)  # k: {E, P} segments of 1024
    FD = wpool.tile([128, N], BF16, tag="FD", name="FD")
    fd2 = FD.rearrange("p (k n) -> p k n", k=2)
    nc.vector.tensor_tensor(
        out=fd2[:, :, :], in0=ep2[:, :, 0:512], in1=ep2[:, :, 512:1024], op=OP.add
    )
    FD2 = wpool.tile([128, N // 2], BF16, tag="FD2", name="FD2")
    fd22 = FD2.rearrange("p (k n) -> p k n", k=2)
    nc.vector.tensor_tensor(
        out=fd22[:, :, :], in0=fd2[:, :, 0:256], in1=fd2[:, :, 256:512], op=OP.add
    )
    nc.vector.tensor_reduce(
        out=t.SS[:, 2 * m:2 * m + 2], in_=fd22[:, :, :],
        axis=mybir.AxisListType.X, op=OP.add,
    )


class _Tiles:
    pass


def _build_program(repeat=1):
    """Build the Bass program (same program for all 8 cores; per-core data
    comes from in_maps). Returns the Bass object. `repeat` re-runs the main
    loop N times inside the NEFF (for slope-based device timing)."""
    nc = bacc.Bacc("TRN2", debug=False, num_devices=1, target_bir_lowering=False)

    # ---- DRAM I/O ----
    dram = {}
    for name, shape, dt in (
        ("xT", [DIM, N], F32), ("xTs", [DIM, ROWS], F32),
        ("pT", [3, N], F32), ("pTs", [3, ROWS], F32),
        ("Wq", [DIM, DIM], F32), ("Wk", [DIM, DIM], F32), ("Wv", [DIM, DIM], F32),
        ("Wpos1", [3, DIM], F32), ("bpos1", [DIM, 1], F32), ("bpos2", [DIM, 1], F32),
        ("Wp2I", [128, DIM], BF16), ("W1dup", [128, HID], BF16),
        ("W2cat", [128, 128], BF16), ("battn1", [128, 2], F32),
    ):
        dram[name] = nc.dram_tensor(name, shape, dt, kind="ExternalInput")
    d_out = nc.dram_tensor("agg_out", [128, NPAIR], F32, kind="ExternalOutput")

    with tile.TileContext(nc) as tc:
        with (
            tc.tile_pool(name="const", bufs=1) as cpool,
            tc.tile_pool(name="work", bufs=4) as wpool,
            tc.tile_pool(name="hsb", bufs=8) as hpool,
        ):
            t = _Tiles()
            # ---------------- persistent SBUF ----------------
            for name, shape, dt in (
                ("xT", [DIM, N], F32), ("xTs", [DIM, ROWS], F32),
                ("pT", [3, N], F32), ("pTs", [3, ROWS], F32),
                ("Wq", [DIM, DIM], F32), ("Wk", [DIM, DIM], F32),
                ("Wv", [DIM, DIM], F32), ("Wpos1", [3, DIM], F32),
                ("bpos1", [DIM, 1], F32), ("bpos2", [DIM, 1], F32),
                ("Wp2I", [128, DIM], BF16), ("W1", [128, HID], BF16),
                ("W2", [128, 128], BF16), ("b1", [128, 2], F32),
                ("vk2", [128, N], BF16), ("bn2", [128, N], BF16),
                ("aA", [DIM, NPAIR], F32), ("aB", [DIM, NPAIR], F32),
                ("qT2", [128, NPAIR], F32), ("qb2", [128, NPAIR], F32),
                ("SS", [128, 2 * NPAIR], F32),
                ("tmp_kT", [DIM, N], F32), ("tmp_vT", [DIM, N], F32),
                ("warm", [128, 8], F32),
                ("recS0", [128, NPAIR], F32), ("agg", [128, NPAIR], F32),
            ):
                setattr(t, name, cpool.tile(shape, dt, tag=name, name=name))
            t.Rbufs = [
                (cpool.tile([128, N], BF16, tag=f"RA{p}", name=f"RA{p}"),
                 cpool.tile([128, N], BF16, tag=f"RB{p}", name=f"RB{p}"))
                for p in range(3)
            ]

            # ---------------- DMA loads ----------------
            for dname, tname in (
                ("xT", "xT"), ("xTs", "xTs"), ("pT", "pT"), ("pTs", "pTs"),
                ("Wq", "Wq"), ("Wk", "Wk"), ("Wv", "Wv"), ("Wpos1", "Wpos1"),
                ("bpos1", "bpos1"), ("bpos2", "bpos2"), ("Wp2I", "Wp2I"),
                ("W1dup", "W1"), ("W2cat", "W2"), ("battn1", "b1"),
            ):
                nc.sync.dma_start(getattr(t, tname)[:], dram[dname].ap())

            # preload the exp table set early (one-time ~2.7us)
            nc.gpsimd.memset(t.warm[:], 0.0)
            nc.scalar.activation(t.warm[:], t.warm[:], AF.Exp)

            _setup_phase(nc, tc, t)

            # ---------------- main loop over row pairs ----------------
            with (
                tc.tile_pool(name="u_ps", bufs=2, space="PSUM") as upool,
                tc.tile_pool(name="h_ps", bufs=2, space="PSUM") as hpspool,
                tc.tile_pool(name="s_ps", bufs=2, space="PSUM") as simpool,
            ):
                pools = (wpool, hpool, upool, hpspool, simpool)
                for _r in range(repeat):
                    for m in range(NPAIR):
                        _pair_iteration(nc, t, pools, m)

            # ---------------- finalize ----------------
            # SS even cols = sum_j E (per pair), odd cols = sum_j E*v_e
            ss2 = t.SS.rearrange("p (m k) -> p m k", k=2)
            nc.vector.reciprocal(t.recS0[:], ss2[:, :, 0])
            nc.vector.tensor_tensor(
                out=t.agg[:], in0=ss2[:, :, 1], in1=t.recS0[:], op=OP.mult
            )
            nc.vector.tensor_tensor(
                out=t.agg[:], in0=t.agg[:], in1=t.qT2[:], op=OP.subtract
            )
            nc.sync.dma_start(d_out.ap(), t.agg[:])

    nc.compile()
    return nc


def _prep_inputs(x, pos, W_qkv, W_pos1, b_pos1, W_pos2, b_pos2,
                 W_attn1, b_attn1, W_attn2, b_attn2):
    """Host-side data prep: slicing/transposes/weight packing (no O(N^2) math)."""
    bf = ml_dtypes.bfloat16
    x2 = np.ascontiguousarray(np.asarray(x, np.float32).reshape(N, DIM))
    p2 = np.ascontiguousarray(np.asarray(pos, np.float32).reshape(N, 3))
    xT = np.ascontiguousarray(x2.T)  # (64, N)
    pT = np.ascontiguousarray(p2.T)  # (3, N)
    W_qkv = np.asarray(W_qkv, np.float32)
    Wq = np.ascontiguousarray(W_qkv[:, 0:DIM])
    Wk = np.ascontiguousarray(W_qkv[:, DIM:2 * DIM])
    Wv = np.ascontiguousarray(W_qkv[:, 2 * DIM:3 * DIM])
    Wp2I = np.concatenate(
        [np.asarray(W_pos2, np.float32), -np.eye(DIM, dtype=np.float32)], axis=0
    ).astype(bf)
    W1dup = np.concatenate(
        [np.asarray(W_attn1, np.float32)] * 2, axis=0
    ).astype(bf)  # (128, 256)
    W2c = np.asarray(W_attn2, np.float32)
    W2cat = np.concatenate([W2c[0:128, :], W2c[128:256, :]], axis=1).astype(bf)
    b1c = np.ascontiguousarray(
        np.asarray(b_attn1, np.float32).reshape(2, 128).T
    )  # (128, 2)
    base = {
        "xT": xT,
        "pT": pT,
        "Wq": Wq, "Wk": Wk, "Wv": Wv,
        "Wpos1": np.ascontiguousarray(np.asarray(W_pos1, np.float32)),
        "bpos1": np.asarray(b_pos1, np.float32).reshape(DIM, 1),
        "bpos2": np.asarray(b_pos2, np.float32).reshape(DIM, 1),
        "Wp2I": Wp2I,
        "W1dup": W1dup,
        "W2cat": W2cat,
        "battn1": b1c,
    }
    in_maps = []
    for c in range(NCORES):
        m = dict(base)
        m["xTs"] = np.ascontiguousarray(xT[:, c * ROWS:(c + 1) * ROWS])
        m["pTs"] = np.ascontiguousarray(pT[:, c * ROWS:(c + 1) * ROWS])
        in_maps.append(m)
    return in_maps


def kernel(x, pos, W_qkv, W_pos1, b_pos1, W_pos2, b_pos2,
           W_attn1, b_attn1, W_attn2, b_attn2, _want_trace=False):
    if "nc" not in _CACHE:
        _CACHE["nc"] = _build_program()
    nc = _CACHE["nc"]
    in_maps = _prep_inputs(x, pos, W_qkv, W_pos1, b_pos1, W_pos2, b_pos2,
                           W_attn1, b_attn1, W_attn2, b_attn2)
    res = run_bass_kernel_spmd(
        nc, in_maps, core_ids=list(range(NCORES)), trace=_want_trace
    )
    _CACHE["last_results"] = res
    out = np.empty((N, DIM), np.float32)
    for c in range(NCORES):
        agg = np.asarray(res.results[c]["agg_out"], np.float32)  # (128, 64)
        out[c * ROWS:c * ROWS + NPAIR, :] = agg[0:DIM, :].T
        out[c * ROWS + NPAIR:(c + 1) * ROWS, :] = agg[DIM:128, :].T
    return out.reshape(1, N, DIM)

